# revision 8
# baseline (speedup 1.0000x reference)
"""Trainium2 Bass kernel for nn_Detector (YOLO-style detector decode).

Contract: kernel(**inputs) takes the FULL unsharded inputs from
setup_inputs() and returns the FULL [340704, 90] fp32 output. Internally
the batch dim (32) is sharded across 8 NeuronCores (4 images per core);
each core decodes its slice of all three scales and the host reassembles
the rows.

Device algorithm (per core), all fp32:
  input x_s [4, 270, HW] is channel-major; output rows (hw, anchor) need
  component-major [row, 90] layout, so each (128-hw-chunk, anchor) block
  is transposed on the TensorEngine ([90ch, <=128hw] -> PSUM [<=128, 90]).
  Per row r: out[r] = mask * [n, sig(p), (dx+ix)t, (dy+iy)t, aw e^dw,
  ah e^dh, point*s, seg(coord*s, sig, sig)...] with
  s = sqrt(w^2+h^2)/case, mask = sig(p) > thresh.

  sigmoid = 0.5*tanh(0.5x)+0.5 so sigmoid+exp share one ScalarE table set
  (exp_and_others); sqrt calls are batched per image to bound set loads.

  fp32 transposes are self-loading matmuls whose LDW struct carries only
  ONE semaphore wait, so ScalarE is the only PSUM reader (one WAR proc)
  and a throwaway 1x1 transpose after each input load absorbs the DMA
  wait before the real transposes issue.
"""
import numpy as np

f32np = np.float32

B = 32
N_CORES = 8
B_LOCAL = B // N_CORES

# (name, W, t, HW)
SCALES = [("52", 52, 8.0, 2704), ("26", 26, 16.0, 676), ("13", 13, 32.0, 169)]
CHUNKS = {name: (HW + 127) // 128 for name, _, _, HW in SCALES}  # 22, 6, 2

# consts column layout: [128, NCONST] fp32
_NTAB = 0                      # 4 cols: n value per local batch
_THR = 4                       # 1 col: thresh
_AW = {"52": 5, "26": 11, "13": 17}    # 6 cols each: (aw,ah) per anchor
_IXY = {"52": 24, "26": 68, "13": 80}  # 2T cols each: (ix,iy) per chunk
NCONST = 84

_CACHE = {}


def _build_nc():
    import concourse.bacc as bacc
    import concourse.tile as tile
    from concourse import mybir
    from concourse.masks import make_identity

    f32 = mybir.dt.float32
    AF = mybir.ActivationFunctionType
    OP = mybir.AluOpType

    # Bacc (not plain Bass): its compile() runs generate_event_semaphores,
    # which legalizes multi-wait instructions down to the 1-wait-per-
    # instruction TRN2 constraint walrus enforces.
    nc = bacc.Bacc("TRN2", target_bir_lowering=False, debug=False)
    xs = {}
    ys = {}
    for name, _, _, HW in SCALES:
        xs[name] = nc.declare_dram_parameter(
            f"x{name}", [B_LOCAL, 270, HW], f32, isOutput=False)
    consts = nc.declare_dram_parameter(
        "consts", [128, NCONST], f32, isOutput=False)
    for name, _, _, HW in SCALES:
        ys[name] = nc.declare_dram_parameter(
            f"y{name}", [B_LOCAL * HW * 3, 90], f32, isOutput=True)

    from concourse.tile_rust import add_dep_helper

    with tile.TileContext(nc) as tc:
        with (
            tc.tile_pool(name="single", bufs=1) as single,
            tc.tile_pool(name="inp", bufs=2) as in_pool,
            tc.tile_pool(name="outp", bufs=4) as out_pool,
            tc.tile_pool(name="small", bufs=6) as small_pool,
            tc.tile_pool(name="psum", bufs=2, space="PSUM") as psum_pool,
            tc.tile_pool(name="psumj", bufs=1, space="PSUM") as psumj_pool,
        ):
            ct = single.tile([128, NCONST], f32)
            nc.gpsimd.dma_start(out=ct[:], in_=consts[:])
            ident = single.tile([128, 128], f32)
            make_identity(nc, ident[:])
            junk = psumj_pool.tile([1, 16], f32)
            # warm-up dummy: absorbs the gpsimd identity wait on PE
            nc.tensor.transpose(junk[0:1, 0:1], ident[0:1, 0:1],
                                ident[0:1, 0:1])
            # last ScalarE PSUM-reader of the previous chunk-group; its wait
            # is pre-pulled onto the next group's final transpose so the
            # first transpose of the following group (which reuses that PSUM
            # slot) never needs a second wait slot (fp32 LDW carries one).
            pend_reader = None

            for b in range(B_LOCAL):
                sc_state = {}
                # ---- phase 1: load + transpose + PSUM evacuation (ACT) ----
                for name, W, t, HW in SCALES:
                    T = CHUNKS[name]
                    G = T * 3
                    in_t = in_pool.tile([90, 3, HW], f32, tag="intile")
                    nc.gpsimd.dma_start(
                        out=in_t[:],
                        in_=xs[name][b].rearrange("(a k) hw -> k a hw", a=3))
                    # throwaway transpose: takes the DMA wait so the real
                    # transposes below each carry at most one (ACT WAR) wait
                    nc.tensor.transpose(junk[0:1, 0:1], in_t[0:1, 0, 0:1],
                                        ident[0:1, 0:1])
                    out_t = out_pool.tile([128, G, 90], f32, tag="outtile")
                    og = out_t[:]

                    for c0 in range(0, T, 4):
                        nch = min(4, T - c0)
                        gcg = nch * 3
                        ps = psum_pool.tile([128, 12, 128], f32, tag="ps")
                        last_tr = None
                        for ci in range(nch):
                            c = c0 + ci
                            w = min(128, HW - c * 128)
                            for a in range(3):
                                last_tr = nc.tensor.transpose(
                                    ps[:w, ci * 3 + a, 0:90],
                                    in_t[:, a, c * 128:c * 128 + w],
                                    ident[0:90, 0:90])
                        if pend_reader is not None:
                            add_dep_helper(
                                last_tr.ins, pend_reader.ins, sync=True,
                                reason="pre-pull next psum slot WAR wait")
                        pg = ps[:, 0:gcg, :]
                        osl = og[:, c0 * 3:c0 * 3 + gcg, :]
                        # seg sigmoids -> tanh(x/2); affine+mask later
                        nc.scalar.activation(
                            osl[:, :, 18:90].rearrange(
                                "p g (i j) -> p g i j", j=3)[:, :, :, 1:3],
                            pg[:, :, 18:90].rearrange(
                                "p g (i j) -> p g i j", j=3)[:, :, :, 1:3],
                            AF.Tanh, scale=0.5)
                        # objectness sigmoid -> tanh(x/2) into col 1
                        nc.scalar.activation(
                            osl[:, :, 1], pg[:, :, 0], AF.Tanh, scale=0.5)
                        # exp(dw), exp(dh) into cols 4,5
                        nc.scalar.activation(
                            osl[:, :, 4:6], pg[:, :, 3:5], AF.Exp)
                        # raw copies: dx,dy | point block | seg coords
                        nc.scalar.copy(osl[:, :, 2:4], pg[:, :, 1:3])
                        nc.scalar.copy(osl[:, :, 6:18], pg[:, :, 6:18])
                        pend_reader = nc.scalar.copy(
                            osl[:, :, 18:90:3], pg[:, :, 18:90:3])
                    sc_state[name] = (out_t, og, G, T)

                # ---- phase 2: sigmoid affine, mask, cx/cy, w/h, q ----
                for name, W, t, HW in SCALES:
                    out_t, og, G, T = sc_state[name]
                    og4 = out_t[:].rearrange("p (c a) k -> p c a k", a=3)
                    segsig = og[:, :, 18:90].rearrange(
                        "p g (i j) -> p g i j", j=3)[:, :, :, 1:3]
                    nc.vector.tensor_scalar(og[:, :, 1], og[:, :, 1],
                                            0.5, 0.5, op0=OP.mult, op1=OP.add)
                    nc.vector.tensor_scalar(segsig, segsig,
                                            0.5, 0.5, op0=OP.mult, op1=OP.add)
                    mask_t = small_pool.tile([128, G], f32, tag="mask")
                    nc.vector.tensor_scalar(
                        mask_t[:], og[:, :, 1], ct[:, _THR:_THR + 1], None,
                        op0=OP.is_gt)
                    # cx,cy: add grid indices then scale by t*mask
                    ixyo = _IXY[name]
                    nc.vector.tensor_add(
                        og4[:, :, :, 2:4], og4[:, :, :, 2:4],
                        ct[:, ixyo:ixyo + 2 * T].rearrange(
                            "p (c k) -> p c k", k=2).unsqueeze(2).broadcast_to(
                                (128, T, 3, 2)))
                    tm_t = small_pool.tile([128, G], f32, tag="tm")
                    nc.vector.tensor_scalar_mul(tm_t[:], mask_t[:], float(t))
                    nc.vector.tensor_mul(
                        og[:, :, 2:4], og[:, :, 2:4],
                        tm_t[:].unsqueeze(2).broadcast_to((128, G, 2)))
                    awo = _AW[name]
                    nc.vector.tensor_mul(
                        og4[:, :, :, 4:6], og4[:, :, :, 4:6],
                        ct[:, awo:awo + 6].rearrange(
                            "p (a w) -> p a w", w=2).unsqueeze(1).broadcast_to(
                                (128, T, 3, 2)))
                    sq_t = small_pool.tile([128, G, 2], f32, tag="sq")
                    nc.vector.tensor_mul(sq_t[:], og[:, :, 4:6], og[:, :, 4:6])
                    q_t = small_pool.tile([128, G], f32, tag="q")
                    nc.vector.tensor_add(q_t[:], sq_t[:, :, 0], sq_t[:, :, 1])
                    sc_state[name] += (mask_t, q_t)

                # ---- phase 3: batched sqrt (one table-set switch) ----
                for name, W, t, HW in SCALES:
                    out_t, og, G, T, mask_t, q_t = sc_state[name]
                    s_t = small_pool.tile([128, G], f32, tag="s")
                    nc.scalar.activation(s_t[:], q_t[:], AF.Sqrt,
                                         scale=1.0 / (416.0 * 416.0))
                    sc_state[name] += (s_t,)

                # ---- phase 4: scale/mask application + store ----
                for name, W, t, HW in SCALES:
                    out_t, og, G, T, mask_t, q_t, s_t = sc_state[name]
                    sm_t = small_pool.tile([128, G], f32, tag="sm")
                    nc.vector.tensor_mul(sm_t[:], s_t[:], mask_t[:])
                    nc.vector.tensor_mul(
                        og[:, :, 6:18], og[:, :, 6:18],
                        sm_t[:].unsqueeze(2).broadcast_to((128, G, 12)))
                    nc.vector.tensor_mul(
                        og[:, :, 18:90:3], og[:, :, 18:90:3],
                        sm_t[:].unsqueeze(2).broadcast_to((128, G, 24)))
                    nc.vector.tensor_mul(og[:, :, 1], og[:, :, 1], mask_t[:])
                    nc.vector.tensor_mul(
                        og[:, :, 4:6], og[:, :, 4:6],
                        mask_t[:].unsqueeze(2).broadcast_to((128, G, 2)))
                    segsig = og[:, :, 18:90].rearrange(
                        "p g (i j) -> p g i j", j=3)[:, :, :, 1:3]
                    nc.vector.tensor_mul(
                        segsig, segsig,
                        mask_t[:].unsqueeze(2).unsqueeze(3).broadcast_to(
                            (128, G, 24, 2)))
                    nc.vector.tensor_mul(
                        og[:, :, 0], mask_t[:],
                        ct[:, _NTAB + b:_NTAB + b + 1].broadcast_to((128, G)))

                    y = ys[name]
                    base = b * HW * 3
                    Tf, rem = HW // 128, HW % 128
                    ov = out_t[:].rearrange("p (c a) k -> p c a k", a=3)
                    if Tf:
                        nc.gpsimd.dma_start(
                            out=y[base:base + Tf * 128 * 3].rearrange(
                                "(c p a) k -> p c a k", p=128, a=3),
                            in_=ov[:, 0:Tf, :, :])
                    if rem:
                        nc.gpsimd.dma_start(
                            out=y[base + Tf * 128 * 3:base + HW * 3].rearrange(
                                "(p a) k -> p a k", a=3),
                            in_=ov[0:rem, Tf, :, :])
    nc.compile()
    return nc


def _host_consts(core, anchors, thresh):
    ct = np.zeros((128, NCONST), f32np)
    for b in range(B_LOCAL):
        ct[:, _NTAB + b] = f32np(core * B_LOCAL + b)
    ct[:, _THR] = f32np(thresh[0])
    for name, W, t, HW in SCALES:
        a = anchors[name].astype(f32np)  # [3, 2]
        ct[:, _AW[name]:_AW[name] + 6] = a.reshape(-1)[None, :]
        T = CHUNKS[name]
        hw = np.arange(T)[None, :] * 128 + np.arange(128)[:, None]  # [128, T]
        o = _IXY[name]
        ct[:, o:o + 2 * T:2] = (hw % W).astype(f32np)
        ct[:, o + 1:o + 2 * T:2] = (hw // W).astype(f32np)
    return ct


def kernel(out13, out26, out52, anchors13, anchors26, anchors52, thresh,
           case, **kw):
    from concourse.bass_utils import run_bass_kernel_spmd

    xs_full = {
        "13": np.ascontiguousarray(np.asarray(out13, f32np).reshape(B, 270, 169)),
        "26": np.ascontiguousarray(np.asarray(out26, f32np).reshape(B, 270, 676)),
        "52": np.ascontiguousarray(np.asarray(out52, f32np).reshape(B, 270, 2704)),
    }
    anchors = {"13": np.asarray(anchors13), "26": np.asarray(anchors26),
               "52": np.asarray(anchors52)}
    thresh = np.asarray(thresh, f32np)

    if "nc" not in _CACHE:
        _CACHE["nc"] = _build_nc()
    nc = _CACHE["nc"]

    in_maps = []
    for core in range(N_CORES):
        bs = slice(core * B_LOCAL, (core + 1) * B_LOCAL)
        m = {f"x{name}": np.ascontiguousarray(xs_full[name][bs])
             for name in ("13", "26", "52")}
        m["consts"] = _host_consts(core, anchors, thresh)
        in_maps.append(m)

    res = run_bass_kernel_spmd(nc, in_maps, list(range(N_CORES))).results

    rows = {name: B * HW * 3 for name, _, _, HW in SCALES}
    out = np.empty((rows["13"] + rows["26"] + rows["52"], 90), f32np)
    off26 = rows["13"]
    off52 = rows["13"] + rows["26"]
    for core in range(N_CORES):
        r = res[core]
        n13 = B_LOCAL * 169 * 3
        n26 = B_LOCAL * 676 * 3
        n52 = B_LOCAL * 2704 * 3
        out[core * n13:(core + 1) * n13] = r["y13"]
        out[off26 + core * n26:off26 + (core + 1) * n26] = r["y26"]
        out[off52 + core * n52:off52 + (core + 1) * n52] = r["y52"]
    return out
